# revision 1
# baseline (speedup 1.0000x reference)
"""Masked dot-product attention (B=16, S=4096, D=64) on 8 Trainium2 NeuronCores.

Decomposition: query-block sharding. Core c owns query rows [c*512, (c+1)*512)
of ALL batches. Every core runs the identical SPMD program: for each batch b it
loops over exactly kb[b] = ceil(valid_lens[b]/128) key-chunks (compile-time
constants derived from the valid_lens input on the host), so masked-out key
blocks are never computed and the load is perfectly balanced across cores.

Math (per batch b, per core c):
  S^T[k,q] = K_chunk[k,:] @ Q[q,:]^T / sqrt(D)      (TensorE, bf16, k on partitions)
  P^T      = exp(S^T)                               (ScalarE, no max-subtraction:
                                                     scores ~ N(0,1), no overflow)
  Oaug^T[65,q] += V_aug_chunk[k,:]^T @ P^T[k,q]     (TensorE, accumulate in PSUM)
where V_aug = [V | 1], with rows k >= valid_len zeroed on the host. The zeroed
rows make masking exact: masked keys contribute 0 to both the numerator and the
ones-column denominator. Host divides numerator by denominator at the end
(exactly softmax @ V, since exp(-1e6 + s) underflows to 0 in fp32 in the
reference as well).
"""

import numpy as np
import ml_dtypes

import concourse.bacc as bacc
import concourse.tile as tile
from concourse import mybir
from concourse.bass_utils import run_bass_kernel_spmd

BF16 = ml_dtypes.bfloat16
F32 = np.float32

NCORES = 8
CH = 128   # key-chunk size (PSUM/PE partition dim)
GRP = 3    # key-chunks per exp() group: 3 PSUM banks, double-buffered = 6 banks
EW = 65    # V_aug width: 64 value dims + 1 ones-column (softmax denominator)


def _schedule(valid_lens, S):
    vl = np.asarray(valid_lens).astype(np.int64)
    vl = np.clip(vl, 1, S)
    kb = [int(-(-int(x) // CH)) for x in vl]          # ceil(valid/CH), >= 1
    pairs = [(x + 1) // 2 for x in kb]
    return vl, kb, pairs


def _build_program(kb, pairs, B, QB, D):
    """Emit the SPMD Tile program. Identical on all cores; per-core data differs."""
    TOT = sum(kb)
    TP = sum(pairs)
    dt = mybir.dt
    nc = bacc.Bacc(None, target_bir_lowering=False)

    kt2 = nc.declare_dram_parameter("kt2", [128, TP * CH], dt.bfloat16, False)
    va = nc.declare_dram_parameter("va", [128, TOT * EW], dt.bfloat16, False)
    qt2 = nc.declare_dram_parameter("qt2", [128, B * QB], dt.bfloat16, False)
    oaug = nc.declare_dram_parameter("oaug", [B, EW, QB], dt.float32, True)

    with tile.TileContext(nc) as tc:
        with (
            tc.tile_pool(name="ins", bufs=1) as ins,
            tc.tile_pool(name="ptp", bufs=3) as ptp,
            tc.tile_pool(name="obp", bufs=3) as obp,
            tc.tile_pool(name="scp", bufs=2, space="PSUM") as scp,
            tc.tile_pool(name="acp", bufs=2, space="PSUM") as acp,
        ):
            kts, vas, qts = [], [], []
            poff = coff = 0
            for b in range(B):
                kt_t = ins.tile([128, pairs[b] * CH], dt.bfloat16, tag=f"kt{b}")
                nc.sync.dma_start(
                    out=kt_t[:], in_=kt2[:, poff * CH:(poff + pairs[b]) * CH]
                )
                va_t = ins.tile([128, kb[b] * EW], dt.bfloat16, tag=f"va{b}")
                nc.sync.dma_start(
                    out=va_t[:], in_=va[:, coff * EW:(coff + kb[b]) * EW]
                )
                qt_t = ins.tile([128, QB], dt.bfloat16, tag=f"qt{b}")
                nc.sync.dma_start(out=qt_t[:], in_=qt2[:, b * QB:(b + 1) * QB])
                kts.append(kt_t)
                vas.append(va_t)
                qts.append(qt_t)
                poff += pairs[b]
                coff += kb[b]

            for b in range(B):
                acc = acp.tile([128, QB], dt.float32, tag="acc")
                nch = kb[b]
                for g0 in range(0, nch, GRP):
                    n = min(GRP, nch - g0)
                    sc = scp.tile([128, GRP, QB], dt.float32, tag="sc")
                    for i in range(n):
                        ci = g0 + i
                        pj, par = divmod(ci, 2)
                        lhsT = kts[b][par * 64:(par + 1) * 64, pj * CH:(pj + 1) * CH]
                        rhs = qts[b][par * 64:(par + 1) * 64, :]
                        nc.tensor.matmul(
                            sc[:, i, :], lhsT, rhs,
                            start=True, stop=True,
                            tile_position=(par * 64, 0),
                        )
                    pt = ptp.tile([128, GRP, QB], dt.bfloat16, tag="pt")
                    nc.scalar.activation(
                        pt[:, :n, :], sc[:, :n, :],
                        mybir.ActivationFunctionType.Exp,
                        scale=float(1.0 / np.sqrt(D)),
                    )
                    for i in range(n):
                        ci = g0 + i
                        nc.tensor.matmul(
                            acc[0:EW, :],
                            vas[b][:, ci * EW:(ci + 1) * EW],
                            pt[:, i, :],
                            start=(ci == 0),
                            stop=(ci == nch - 1),
                        )
                ob = obp.tile([128, QB], dt.float32, tag="ob")
                nc.vector.tensor_copy(ob[0:EW, :], acc[0:EW, :])
                nc.sync.dma_start(out=oaug[b], in_=ob[0:EW, :])

    nc.compile()
    return nc


def _prepare(q, k, v, valid_lens):
    """Host-side sharding/layout. Returns (nc, in_maps, meta)."""
    q = np.asarray(q, dtype=F32)
    k = np.asarray(k, dtype=F32)
    v = np.asarray(v, dtype=F32)
    B, S, D = q.shape
    QB = S // NCORES
    vl, kb, pairs = _schedule(valid_lens, S)
    TOT, TP = sum(kb), sum(pairs)

    # kt2: [128, TP*CH] bf16. Pair j of batch b: partitions 0:64 <- K^T chunk 2j,
    # partitions 64:128 <- K^T chunk 2j+1 (left zero if absent). Concurrent
    # row-group matmuls on the PE use both halves of the systolic array.
    kT = np.ascontiguousarray(k.transpose(0, 2, 1)).astype(BF16)  # [B, D, S]
    kt2 = np.zeros((128, TP * CH), dtype=BF16)
    poff = 0
    for b in range(B):
        for j in range(pairs[b]):
            c0, c1 = 2 * j, 2 * j + 1
            kt2[0:64, (poff + j) * CH:(poff + j + 1) * CH] = \
                kT[b][:, c0 * CH:(c0 + 1) * CH]
            if c1 < kb[b]:
                kt2[64:128, (poff + j) * CH:(poff + j + 1) * CH] = \
                    kT[b][:, c1 * CH:(c1 + 1) * CH]
        poff += pairs[b]

    # va: [128, TOT*EW] bf16. Chunk g of batch b at columns (coff+g)*EW:
    # va[p, (coff+g)*EW + e] = V_aug[b, g*CH + p, e], rows >= valid zeroed.
    va_aug = np.zeros((B, S, EW), dtype=F32)
    va_aug[:, :, :D] = v
    va_aug[:, :, D] = 1.0
    for b in range(B):
        va_aug[b, int(vl[b]):, :] = 0.0
    va_aug = va_aug.astype(BF16)
    va = np.zeros((128, TOT * EW), dtype=BF16)
    coff = 0
    for b in range(B):
        blk = va_aug[b, :kb[b] * CH, :].reshape(kb[b], CH, EW)
        va[:, coff * EW:(coff + kb[b]) * EW] = \
            blk.transpose(1, 0, 2).reshape(CH, kb[b] * EW)
        coff += kb[b]

    # qt2 (per core): [128, B*QB] bf16, Q^T slice duplicated on both partition
    # halves (each PE row-group streams its own rhs copy).
    qT = np.ascontiguousarray(q.transpose(0, 2, 1)).astype(BF16)  # [B, D, S]
    in_maps = []
    for c in range(NCORES):
        qt2 = np.zeros((128, B * QB), dtype=BF16)
        for b in range(B):
            sl = qT[b][:, c * QB:(c + 1) * QB]
            qt2[0:64, b * QB:(b + 1) * QB] = sl
            qt2[64:128, b * QB:(b + 1) * QB] = sl
        in_maps.append({"kt2": kt2, "va": va, "qt2": qt2})

    nc = _build_program(kb, pairs, B, QB, D)
    return nc, in_maps, (B, S, D, QB)


def _postprocess(results, meta):
    B, S, D, QB = meta
    out = np.empty((B, S, D), dtype=F32)
    for c in range(NCORES):
        oa = results[c]["oaug"]          # [B, EW, QB] f32
        num = oa[:, :D, :]
        den = oa[:, D:D + 1, :]
        out[:, c * QB:(c + 1) * QB, :] = (num / den).transpose(0, 2, 1)
    return out


def kernel(q, k, v, valid_lens):
    nc, in_maps, meta = _prepare(q, k, v, valid_lens)
    res = run_bass_kernel_spmd(nc, in_maps, list(range(NCORES)))
    return _postprocess(res.results, meta)
